# revision 7
# baseline (speedup 1.0000x reference)
"""Trainium2 Bass kernel for nn_DisplacedGTOExternalFieldBlock (v8).

v5 (engine-split gather-as-GEMM) + S=352 block segments (third node-tile
is 96 wide; ~8.3% less mask/compare/matmul/output work; ~0.08% of nodes
overflow to exact host fixup) + deeper mask buffering (bufs=3).

Lanes per 16-block superblock (SBW=5632 mask cols):
- Pool lane (2 of 3): partition_broadcast (2 chunks) -> DVE is_equal (4
  chunks of 1408) from SBUF.
- PE lane (1 of 3): row0-ones matmul broadcast into PSUM (11 x 512) ->
  DVE is_equal from PSUM (4 chunks).
Matmuls are mask-stationary [128,{128,128,96}] @ [128,32] -> PSUM banks of
16 slots; Scalar engine evacuates full banks to bf16; ~206GB/s out DMA.
"""

import numpy as np
import ml_dtypes

import concourse.bass as bass
import concourse.bacc as bacc
import concourse.mybir as mybir
import concourse.tile as tile
from concourse.bass_utils import run_bass_kernel_spmd

N_NODES = 2_000_000
N_GRAPHS = 100_000
P_OUT = 32
N_CORES = 8
PER_CORE = N_NODES // N_CORES
PART = 128

NBLK = 784
S_DEV = 352  # device columns per block (mean 319, +1.85 sigma)
S_PI = 384  # decode row stride per block (3 tiles x 128 partitions)
TILE_OFF = (0, 128, 256)
TILE_W = (128, 128, 128)  # third tile reads 32 cols into next block / zero tail
BLK_SB = 16
N_SB = NBLK // BLK_SB  # 49
SBW = BLK_SB * S_DEV  # 5632
TILES_SB = 48
CH = 4  # compare chunks (4 blocks = 1408 cols each)
CHW = SBW // CH  # 1408
BCMM = SBW // 512  # 11 ones-matmul chunks in PE lane

DMA_BC = 2  # sb % 2 == 0 -> host-replicated broadcast via DMA (SBUF compare
# hits the DVE fast path; PSUM-sourced compares run ~4x slower)

BF16 = mybir.dt.bfloat16
F32 = mybir.dt.float32

_NC_CACHE = {}


def _build_nc():
    nc = bacc.Bacc("TRN2", target_bir_lowering=False)
    tab_d = nc.dram_tensor("tab", [PART, NBLK * P_OUT], BF16, kind="ExternalInput")
    loc_d = nc.dram_tensor("loc", [N_SB, SBW], BF16, kind="ExternalInput")
    iota_d = nc.dram_tensor("iota", [PART, 1], F32, kind="ExternalInput")
    ones_d = nc.dram_tensor("ones", [PART, PART], BF16, kind="ExternalInput")
    bcd_d = nc.dram_tensor(
        "bcd", [(N_SB + 1) // DMA_BC, PART, SBW], BF16, kind="ExternalInput"
    )
    out_d = nc.dram_tensor(
        "out", [N_SB, PART, TILES_SB * P_OUT], BF16, kind="ExternalOutput"
    )

    with tile.TileContext(nc) as tc:
        with (
            tc.tile_pool(name="cst", bufs=1) as cpool,
            tc.tile_pool(name="lp", bufs=2) as lpool,
            tc.tile_pool(name="bcp", bufs=2) as bcpool,
            tc.tile_pool(name="mkp", bufs=3) as mkpool,
            tc.tile_pool(name="pbc", bufs=2, space="PSUM") as pbcpool,
            tc.tile_pool(name="pob", bufs=2, space="PSUM") as pobpool,
            tc.tile_pool(name="obp", bufs=2) as obpool,
        ):
            tab_s = cpool.tile([PART, NBLK * P_OUT], BF16, tag="tab")
            nc.scalar.dma_start(out=tab_s[:], in_=tab_d[:])
            iota_s = cpool.tile([PART, 1], F32, tag="iota")
            nc.sync.dma_start(out=iota_s[:], in_=iota_d[:])
            ones_s = cpool.tile([PART, PART], BF16, tag="ones")
            nc.sync.dma_start(out=ones_s[:], in_=ones_d[:])
            xqs = []
            for i in range(2):
                xq = cpool.tile([PART, SBW], BF16, tag=f"xq{i}")
                nc.vector.memset(xq[:], 0.0)
                xqs.append(xq)

            pe_i = 0
            for sb in range(N_SB):
                dma_lane = sb % DMA_BC == 0
                mk = mkpool.tile([PART, SBW + 32], BF16, tag="mk")
                # zero tail so the last 128-wide lhsT reads defined zeros
                nc.vector.memset(mk[:, SBW : SBW + 32], 0.0)
                if dma_lane:
                    bc = bcpool.tile([PART, SBW], BF16, tag="bc")
                    eng = nc.scalar if (sb // DMA_BC) % 2 == 0 else nc.sync
                    eng.dma_start(out=bc[:], in_=bcd_d[sb // DMA_BC])
                    for ch in range(CH):
                        nc.vector.tensor_scalar(
                            out=mk[:, ch * CHW : (ch + 1) * CHW],
                            in0=bc[:, ch * CHW : (ch + 1) * CHW],
                            scalar1=iota_s[:],
                            scalar2=None,
                            op0=mybir.AluOpType.is_equal,
                        )
                else:
                    xq = xqs[pe_i % 2]
                    pe_i += 1
                    nc.sync.dma_start(out=xq[0:1, :], in_=loc_d[sb : sb + 1, :])
                    # 11 x 512-col ones-matmuls into 4 PSUM tiles of 1408
                    # (1408 = 2.75 banks -> allocate [128, 1536] 3 banks,
                    # matmul chunks must stay within banks: use 1408 = 512+512+384)
                    for ch in range(CH):
                        bcp = pbcpool.tile([PART, 1536], F32, tag="bcp")
                        base = ch * CHW
                        off = 0
                        for w in (512, 512, 384):
                            nc.tensor.matmul(
                                out=bcp[:, off : off + w],
                                lhsT=ones_s[:],
                                rhs=xq[:, base + off : base + off + w],
                                start=True,
                                stop=True,
                            )
                            off += w
                        nc.vector.tensor_scalar(
                            out=mk[:, base : base + CHW],
                            in0=bcp[:, 0:CHW],
                            scalar1=iota_s[:],
                            scalar2=None,
                            op0=mybir.AluOpType.is_equal,
                        )
                ob = obpool.tile([PART, TILES_SB * P_OUT], BF16, tag="ob")
                for h in range(3):
                    ps = pobpool.tile([PART, 512], F32, tag="ps")
                    for s in range(16):
                        t = 16 * h + s
                        b16, tt = t // 3, t % 3
                        col = b16 * S_DEV + TILE_OFF[tt]
                        w = TILE_W[tt]
                        nc.tensor.matmul(
                            out=ps[:, 32 * s : 32 * s + 32],
                            lhsT=mk[:, col : col + w],
                            rhs=tab_s[
                                :,
                                P_OUT * (sb * BLK_SB + b16) : P_OUT
                                * (sb * BLK_SB + b16 + 1),
                            ],
                            start=True,
                            stop=True,
                        )
                    nc.scalar.copy(out=ob[:, 512 * h : 512 * (h + 1)], in_=ps[:])
                nc.sync.dma_start(out=out_d[sb], in_=ob[:])
    nc.compile()
    return nc


def _get_nc():
    if "nc" not in _NC_CACHE:
        _NC_CACHE["nc"] = _build_nc()
    return _NC_CACHE["nc"]


def _prep_core(idx32):
    order = np.argsort(idx32, kind="stable")
    sidx = idx32[order]
    blk = (sidx >> 7).astype(np.int64)
    loc = (sidx & 127).astype(ml_dtypes.bfloat16)
    counts = np.bincount(blk, minlength=NBLK)
    starts = np.zeros(NBLK, dtype=np.int64)
    np.cumsum(counts[:-1], out=starts[1:])
    j = np.arange(PER_CORE, dtype=np.int64) - starts[blk]
    sel = j < S_DEV
    locd = np.full((NBLK, S_DEV), -1.0, dtype=ml_dtypes.bfloat16)
    locd[blk[sel], j[sel]] = loc[sel]
    pi = np.full(NBLK * S_PI, -1, dtype=np.int64)
    pi[blk[sel] * S_PI + j[sel]] = order[sel]
    ovf_pos = order[~sel]
    return locd.reshape(N_SB, SBW), pi, ovf_pos


def kernel(batch, positions, field, matrix):
    return run(batch, positions, field, matrix)[0]


def run(batch, positions, field, matrix, trace=False, trace_cores=None):
    del positions
    batch = np.ascontiguousarray(np.asarray(batch, dtype=np.int32))
    field = np.ascontiguousarray(np.asarray(field, dtype=np.float32))
    matrix = np.asarray(matrix, dtype=np.float32)
    assert batch.shape == (N_NODES,)
    assert field.shape == (N_GRAPHS, 4)
    assert matrix.shape == (P_OUT, 4)

    meff = matrix[:, [0, 2, 3, 1]]
    proj = np.ascontiguousarray(field @ meff.T)
    proj_pad = np.zeros((NBLK * PART, P_OUT), dtype=np.float32)
    proj_pad[:N_GRAPHS] = proj
    tab = np.ascontiguousarray(
        proj_pad.reshape(NBLK, PART, P_OUT)
        .transpose(1, 0, 2)
        .reshape(PART, NBLK * P_OUT)
        .astype(ml_dtypes.bfloat16)
    )
    iota = np.arange(PART, dtype=np.float32).reshape(PART, 1)
    ones = np.zeros((PART, PART), dtype=ml_dtypes.bfloat16)
    ones[0, :] = 1.0

    nc = _get_nc()
    in_maps = []
    pis = []
    ovfs = []
    for c in range(N_CORES):
        idx_c = batch[c * PER_CORE : (c + 1) * PER_CORE]
        locd, pi, ovf = _prep_core(idx_c)
        bcd = np.ascontiguousarray(
            np.broadcast_to(
                locd[0::DMA_BC][:, None, :],
                ((N_SB + 1) // DMA_BC, PART, SBW),
            )
        )
        in_maps.append(
            {"tab": tab, "loc": locd, "iota": iota, "ones": ones, "bcd": bcd}
        )
        pis.append(pi)
        ovfs.append(ovf)

    kwargs = {}
    if trace:
        kwargs["trace"] = True
        if trace_cores is not None:
            kwargs["trace_cores"] = trace_cores
    res = run_bass_kernel_spmd(nc, in_maps, core_ids=list(range(N_CORES)), **kwargs)

    out = np.empty((N_NODES, P_OUT), dtype=np.float32)
    for c in range(N_CORES):
        pi = pis[c]
        valid = pi >= 0
        # dev row (sb, t, p) -> block b = sb*16 + t//3, j = (t%3)*128 + p
        dev = (
            np.asarray(res.results[c]["out"])
            .reshape(N_SB, PART, 3, 16, P_OUT)
            .transpose(0, 2, 3, 1, 4)  # [sb, h, s, p, f]
            .reshape(NBLK * S_PI, P_OUT)
            .astype(np.float32)
        )
        out[c * PER_CORE + pi[valid]] = dev[valid]
        ovf = ovfs[c]
        if len(ovf):
            out[c * PER_CORE + ovf] = proj[batch[c * PER_CORE + ovf]]
    return out, res
